# revision 8
# baseline (speedup 1.0000x reference)
"""GroupedQueryAttention (B=1, T=4096, D=2048, 16 q-heads / 4 kv-heads, RoPE,
causal) on 8 Trainium2 NeuronCores — v6.

Sharding: head tensor-parallel, core c owns q-heads {2c, 2c+1} and kv head
c//2. Each core computes partial out = ctx_heads @ WO_slice over the full
sequence; bf16 partials are summed on the host.

v6 idea: ONE dense tensor-engine stream. All chunk-boundary PE work
(projection accums, RoPE perm, V transposes, normalization broadcast, WO
output tiles) is queued as filler closures and drained inside the
ACT-paced attention loops, so the PE never ping-pongs between a PE-only
projection window and an ACT-bound attention window (which left HAM
oscillating at low clock).

PSUM: psX 3x[128,512] (S tiles, attention-only) + psC 2 (ctx per head)
+ psL 1 (both heads' denominators at partitions 0/32, col-packed ones
matmuls) + psW 2 (all transient boundary tiles) = 8 banks.
"""

import math

import numpy as np
import ml_dtypes

import concourse.bass as bass
import concourse.mybir as mybir
import concourse.tile as tile
from concourse.bass_utils import run_bass_kernel_spmd

FP = mybir.dt.float32
BF = mybir.dt.bfloat16
BFNP = ml_dtypes.bfloat16

T, D, DH = 4096, 2048, 128      # seq len, model dim, head dim
CH = 512                        # query-chunk (free dim of attention matmuls)
N_CORES = 8


# --------------------------------------------------------------------------
# workaround: this walrus build rejects instructions carrying >1 sem-waits
# (setupSyncWait "Too many sync wait commands"); split extras into NoOps.
_WS_CTR = [0]


def _split_multi_waits(nc, limit=1):
    for f in nc.m.functions:
        for bb in f.blocks:
            il = bb.instructions
            i = 0
            while i < len(il):
                inst = il[i]
                si = getattr(inst, "sync_info", None)
                if si is not None and len(si.on_wait) > limit:
                    waits = list(si.on_wait)
                    keep, rest = waits[:limit], waits[limit:]
                    nops = []
                    for j in range(0, len(rest), limit):
                        _WS_CTR[0] += 1
                        n = mybir.InstNoOp(name=f"waitsplit-{_WS_CTR[0]}")
                        n.engine = inst.engine
                        n.sync_info = mybir.SyncInfo(
                            on_wait=rest[j:j + limit], on_update=[])
                        nops.append(n)
                    inst.sync_info = mybir.SyncInfo(
                        on_wait=keep, on_update=list(si.on_update))
                    for k, n in enumerate(nops):
                        il.insert(i + k, n)
                    i += len(nops)
                i += 1


# --------------------------------------------------------------------------
def build_nc():
    nT = T // CH          # 8 T-chunks
    nA = D // 128         # 16 contraction tiles
    nS = CH // 128        # 4 kt subtiles per chunk
    nN = D // 512         # 4 output column tiles
    ISQ = 1.0 / math.sqrt(float(DH))

    nc = bass.Bass()

    xT = nc.dram_tensor("xT", [D, T], BF, kind="ExternalInput")
    wq2 = nc.dram_tensor("wq2", [128, nA * 256], BF, kind="ExternalInput")
    wk2 = nc.dram_tensor("wk2", [128, nA * 128], BF, kind="ExternalInput")
    wv2 = nc.dram_tensor("wv2", [128, nA * 128], BF, kind="ExternalInput")
    wo2 = nc.dram_tensor("wo2", [128, 2 * D], BF, kind="ExternalInput")
    cosT = nc.dram_tensor("cosT", [128, T], BF, kind="ExternalInput")
    sinT = nc.dram_tensor("sinT", [128, T], BF, kind="ExternalInput")
    permM = nc.dram_tensor("permM", [128, 128], BF, kind="ExternalInput")
    masks = nc.dram_tensor("masks", [128, nS * CH], BF, kind="ExternalInput")
    onescol = nc.dram_tensor("onescol", [128, 1], BF, kind="ExternalInput")
    onesrow = nc.dram_tensor("onesrow", [64, 128], BF, kind="ExternalInput")
    ident = nc.dram_tensor("ident", [128, 128], BF, kind="ExternalInput")
    out = nc.dram_tensor("out", [T, D], BF, kind="ExternalOutput")

    xTr = xT.rearrange("(a p) t -> p a t", p=128)

    with tile.TileContext(nc) as tc:
        with (
            tc.tile_pool(name="res", bufs=1) as res,
            tc.tile_pool(name="ktv", bufs=2 * nT) as ktv,
            tc.tile_pool(name="xt", bufs=32) as xtp,
            tc.tile_pool(name="qk", bufs=4) as qkp,     # q/k/vT sbuf copies
            tc.tile_pool(name="qr", bufs=4) as qrp,     # rope outputs q heads
            tc.tile_pool(name="rt", bufs=4) as rtp,     # rope temporaries
            tc.tile_pool(name="pp", bufs=6) as ppp,     # P tiles
            tc.tile_pool(name="nrm", bufs=2) as nrm,    # recip / bcast / cn
            tc.tile_pool(name="oo", bufs=4) as oop,     # out staging bf16
            tc.tile_pool(name="psX", bufs=3, space="PSUM") as psX,
            tc.tile_pool(name="psC", bufs=2, space="PSUM") as psC,
            tc.tile_pool(name="psL", bufs=1, space="PSUM") as psL,
            tc.tile_pool(name="psW", bufs=2, space="PSUM") as psW,
        ):
            wq_sb = res.tile([128, nA * 256], BF, name="wq_sb")
            wk_sb = res.tile([128, nA * 128], BF, name="wk_sb")
            wv_sb = res.tile([128, nA * 128], BF, name="wv_sb")
            wo_sb = res.tile([128, 2 * D], BF, name="wo_sb")
            mask_sb = res.tile([128, nS * CH], BF, name="mask_sb")
            perm_sb = res.tile([128, 128], BF, name="perm_sb")
            oc_sb = res.tile([128, 1], BF, name="oc_sb")
            or_sb = res.tile([64, 128], BF, name="or_sb")
            id_sb = res.tile([128, 128], BF, name="id_sb")
            cos_sb = res.tile([128, T], BF, name="cos_sb")
            sin_sb = res.tile([128, T], BF, name="sin_sb")

            kt_tiles = {}
            v_tiles = {}
            x_chunks = {}
            qr_chunks = {}

            def load_x(qc):
                t0 = qc * CH
                xts = []
                for a in range(nA):
                    xa = xtp.tile([128, CH], BF, name=f"x{qc}_{a}", tag="xt")
                    nc.sync.dma_start(xa[:], xTr[:, a, t0:t0 + CH])
                    xts.append(xa)
                x_chunks[qc] = xts

            # x for chunk 0 first so projections can start ASAP, then wq,
            # then everything else.
            load_x(0)
            for t_, s_ in [
                (wq_sb, wq2), (wk_sb, wk2), (wv_sb, wv2), (cos_sb, cosT),
                (sin_sb, sinT), (perm_sb, permM), (mask_sb, masks),
                (oc_sb, onescol), (or_sb, onesrow), (id_sb, ident),
                (wo_sb, wo2),
            ]:
                nc.sync.dma_start(t_[:], s_[:])

            # ---------- projection / rope closures ----------
            def proj_closures(qc):
                """Filler closures, in dependency order, computing Q/K/V^T
                projections, V transposes, and RoPE for chunk qc."""
                t0 = qc * CH
                cos_t = cos_sb[:, t0:t0 + CH]
                sin_t = sin_sb[:, t0:t0 + CH]
                qr0 = qrp.tile([128, CH], BF, name=f"qr0_{qc}", tag="qr")
                qr1 = qrp.tile([128, CH], BF, name=f"qr1_{qc}", tag="qr")
                ktt = ktv.tile([128, CH], BF, name=f"kt{qc}", tag="ktv")
                qr_chunks[qc] = (qr0, qr1)
                kt_tiles[qc] = ktt
                vt = ktv.tile([128, nS * 128], BF, name=f"v{qc}", tag="ktv")
                v_tiles[qc] = vt

                sb_holder = {}

                def mk_accum(lhs, nm):
                    def emit():
                        xts = x_chunks[qc]
                        acc = psW.tile([128, CH], FP, name=f"ps_{nm}",
                                       tag="w")
                        for a in range(nA):
                            nc.tensor.matmul(
                                acc[:], lhs(a), xts[a][:],
                                start=(a == 0), stop=(a == nA - 1))
                        sb_ = qkp.tile([128, CH], BF, name=f"{nm}s",
                                       tag="qk")
                        nc.scalar.copy(sb_[:], acc[:])
                        sb_holder[nm] = sb_
                    return emit

                def mk_rope(nm, dst):
                    def emit():
                        src_sb = sb_holder[nm]
                        sw = psW.tile([128, CH], FP, name=f"sw_{nm}",
                                      tag="w")
                        nc.tensor.matmul(sw[:], perm_sb[:], src_sb[:])
                        t1 = rtp.tile([128, CH], BF, name=f"r1_{nm}",
                                      tag="rt")
                        nc.vector.tensor_mul(t1[:], src_sb[:], cos_t)
                        t2 = rtp.tile([128, CH], BF, name=f"r2_{nm}",
                                      tag="rt")
                        nc.vector.tensor_mul(t2[:], sw[:], sin_t)
                        nc.vector.tensor_add(dst[:], t1[:], t2[:])
                    return emit

                def mk_vtrans():
                    def emit():
                        vT_sb = sb_holder[f"vT{qc}"]
                        for s in range(nS):
                            tp = psW.tile([128, 128], BF, name=f"tp{qc}_{s}",
                                          tag="w", padded_shape=[128, 512])
                            nc.tensor.transpose(
                                tp[:], vT_sb[:, s * 128:(s + 1) * 128],
                                id_sb[:])
                            nc.vector.tensor_copy(
                                vt[:, s * 128:(s + 1) * 128], tp[:])
                    return emit

                cl = []
                cl.append(mk_accum(lambda a: wq_sb[:, a * 256:a * 256 + 128],
                                   f"q0{qc}"))
                cl.append(mk_rope(f"q0{qc}", qr0))
                cl.append(mk_accum(lambda a: wq_sb[:, a * 256 + 128:
                                                   a * 256 + 256],
                                   f"q1{qc}"))
                cl.append(mk_rope(f"q1{qc}", qr1))
                cl.append(mk_accum(lambda a: wk_sb[:, a * 128:(a + 1) * 128],
                                   f"k{qc}"))
                cl.append(mk_rope(f"k{qc}", ktt))
                cl.append(mk_accum(lambda a: wv_sb[:, a * 128:(a + 1) * 128],
                                   f"vT{qc}"))
                cl.append(mk_vtrans())
                return cl

            # ---------- normalization part 2 + WO closures ----------
            def norm2_wo_closures(qc, rb_t, ctx):
                t0 = qc * CH
                cns = []

                def mk_norm(h):
                    def emit():
                        bc_ps = psW.tile([128, CH], FP, name=f"bc{qc}_{h}",
                                         tag="w")
                        nc.tensor.matmul(bc_ps[:],
                                         or_sb[32 * h:32 * h + 1, :],
                                         rb_t[32 * h:32 * h + 1, :])
                        bc_sb = nrm.tile([128, CH], FP, name=f"bcs{qc}_{h}",
                                         tag="bc")
                        nc.scalar.copy(bc_sb[:], bc_ps[:])
                        cn = nrm.tile([128, CH], BF, name=f"cn{qc}_{h}",
                                      tag="cn")
                        nc.vector.tensor_mul(cn[:], ctx[h][:], bc_sb[:])
                        cns.append(cn)
                    return emit

                def mk_wo(s, n):
                    def emit():
                        w_ps = psW.tile([128, 512], FP,
                                        name=f"w{qc}_{s}_{n}", tag="w")
                        nc.tensor.matmul(
                            w_ps[:], cns[0][:, s * 128:(s + 1) * 128],
                            wo_sb[:, n * 512:(n + 1) * 512],
                            start=True, stop=False)
                        nc.tensor.matmul(
                            w_ps[:], cns[1][:, s * 128:(s + 1) * 128],
                            wo_sb[:, D + n * 512:D + (n + 1) * 512],
                            start=False, stop=True)
                        osb = oop.tile([128, 512], BF, name=f"o{qc}_{s}_{n}",
                                       tag="osb")
                        if qc == nT - 1 and n % 2:
                            nc.scalar.copy(osb[:], w_ps[:])
                        else:
                            nc.vector.tensor_copy(osb[:], w_ps[:])
                        nc.sync.dma_start(
                            out[t0 + s * 128:t0 + (s + 1) * 128,
                                n * 512:(n + 1) * 512], osb[:])
                    return emit

                return ([mk_norm(0), mk_norm(1)] +
                        [mk_wo(s, n) for s in range(nS) for n in range(nN)])

            # ---------- attention ----------
            def emit_attention(qc, queue):
                """Causal attention for chunk qc; pops filler closures from
                `queue` inside the loop and drains it at the end."""
                nkt = (qc + 1) * nS
                qr0, qr1 = qr_chunks[qc]
                qrs = (qr0, qr1)

                ctx = [psC.tile([128, CH], FP, name=f"ctx{qc}_{h}", tag="ctx")
                       for h in range(2)]
                lt = psL.tile([128, CH], FP, name=f"l{qc}", tag="l")
                lps = [lt[0:1, :], lt[32:33, :]]

                def emit_s(kt, h):
                    kc, ko = kt // nS, (kt % nS) * 128
                    sp = psX.tile([128, CH], FP, name=f"S{qc}_{h}_{kt}",
                                  tag="s")
                    nc.tensor.matmul(sp[:], kt_tiles[kc][:, ko:ko + 128],
                                     qrs[h][:])
                    return sp

                def emit_exp(kt, h, sp):
                    pt = ppp.tile([128, CH], BF, name=f"P{qc}_{h}_{kt}",
                                  tag="p")
                    nc.scalar.activation(pt[:], sp[:],
                                         mybir.ActivationFunctionType.Exp,
                                         scale=ISQ)
                    delta = kt - qc * nS
                    if delta >= 0:  # diagonal chunk: causal mask
                        nc.vector.tensor_mul(
                            pt[:], pt[:],
                            mask_sb[:, delta * CH:(delta + 1) * CH])
                    return pt

                # pipeline: S/exp one step ahead; l pair adjacent (packed)
                p0 = emit_exp(0, 0, emit_s(0, 0))
                p1 = emit_exp(0, 1, emit_s(0, 1))
                for kt in range(nkt):
                    if kt + 1 < nkt:
                        pn0 = emit_exp(kt + 1, 0, emit_s(kt + 1, 0))
                        pn1 = emit_exp(kt + 1, 1, emit_s(kt + 1, 1))
                    st, sp_ = (kt == 0), (kt == nkt - 1)
                    nc.tensor.matmul(lps[0], oc_sb[:], p0[:],
                                     start=st, stop=sp_)
                    nc.tensor.matmul(lps[1], oc_sb[:], p1[:],
                                     start=st, stop=sp_)
                    kc, ko = kt // nS, (kt % nS) * 128
                    vt = v_tiles[kc]
                    nc.tensor.matmul(ctx[0][:], vt[:, ko:ko + 128], p0[:],
                                     start=st, stop=sp_)
                    nc.tensor.matmul(ctx[1][:], vt[:, ko:ko + 128], p1[:],
                                     start=st, stop=sp_)
                    if len(queue) > 5:   # hold a reserve for the chunk end
                        queue.pop(0)()
                    if kt + 1 < nkt:
                        p0, p1 = pn0, pn1

                # normalization part 1 (DVE), emitted BEFORE the drain so the
                # chain starts immediately: copy l out of PSUM (frees the bank
                # for the next chunk's accumulation ~4us earlier than letting
                # the reciprocal hold it), then reciprocal + bf16 cast on the
                # SBUF copy. The reserved fillers below keep the PE busy while
                # the chain completes.
                l_sb = nrm.tile([64, CH], FP, name=f"ls{qc}", tag="ls")
                nc.vector.tensor_copy(l_sb[:], lt[0:64, :])
                r_t = nrm.tile([64, CH], FP, name=f"r{qc}", tag="r")
                nc.vector.reciprocal(r_t[:], l_sb[:])
                rb_t = nrm.tile([64, CH], BF, name=f"rb{qc}", tag="rb")
                nc.vector.tensor_copy(rb_t[:], r_t[:])
                while queue:   # must drain: next chunk needs qr/kt ready
                    queue.pop(0)()
                return rb_t, ctx

            # ---------- main schedule ----------
            queue = []
            for cl in proj_closures(0):
                cl()
            for qc in range(nT):
                if qc + 1 < nT:
                    load_x(qc + 1)
                    queue.extend(proj_closures(qc + 1))
                rb_t, ctx = emit_attention(qc, queue)
                queue.extend(norm2_wo_closures(qc, rb_t, ctx))
            for cl in queue:
                cl()

    _split_multi_waits(nc, 1)
    return nc


# --------------------------------------------------------------------------
def host_prep(x, WQ, WK, WV, WO):
    nA = D // 128
    nS = CH // 128
    ROPE_BASE = 10000.0

    xTc = np.ascontiguousarray(
        np.asarray(x, dtype=np.float32).reshape(T, D).T).astype(BFNP)

    omega = 1.0 / (ROPE_BASE ** (np.arange(0, DH, 2, dtype=np.float64) / DH))
    ang = np.outer(omega, np.arange(T, dtype=np.float64))
    cosT = np.repeat(np.cos(ang), 2, axis=0).astype(BFNP)
    sgn = np.tile(np.array([-1.0, 1.0]), DH // 2)[:, None]
    sinT = (np.repeat(np.sin(ang), 2, axis=0) * sgn).astype(BFNP)

    permM = np.zeros((128, 128), dtype=np.float32)
    for j in range(0, 128, 2):
        permM[j + 1, j] = 1.0
        permM[j, j + 1] = 1.0
    permM = permM.astype(BFNP)

    p_i = np.arange(128)[:, None]
    f_i = np.arange(CH)[None, :]
    masks = np.concatenate(
        [(128 * dl + p_i <= f_i).astype(np.float32) for dl in range(nS)],
        axis=1).astype(BFNP)

    def tile_pmaj(w, ncols):
        return np.ascontiguousarray(
            np.asarray(w, dtype=np.float32).reshape(nA, 128, ncols)
            .transpose(1, 0, 2).reshape(128, nA * ncols)).astype(BFNP)

    in_maps = []
    for c in range(N_CORES):
        kv = c // 2
        wo_c = np.asarray(WO, dtype=np.float32)[256 * c:256 * (c + 1), :]
        in_maps.append({
            "xT": xTc,
            "wq2": tile_pmaj(np.asarray(WQ)[:, 256 * c:256 * (c + 1)], 256),
            "wk2": tile_pmaj(np.asarray(WK)[:, 128 * kv:128 * (kv + 1)], 128),
            "wv2": tile_pmaj(np.asarray(WV)[:, 128 * kv:128 * (kv + 1)], 128),
            "wo2": np.ascontiguousarray(
                wo_c.reshape(2, 128, D).transpose(1, 0, 2)
                .reshape(128, 2 * D)).astype(BFNP),
            "cosT": cosT, "sinT": sinT, "permM": permM, "masks": masks,
            "onescol": np.ones((128, 1), dtype=BFNP),
            "onesrow": np.ones((64, 128), dtype=BFNP),
            "ident": np.eye(128, dtype=np.float32).astype(BFNP),
        })
    return in_maps


_NC_CACHE = {}


def _get_nc():
    if "nc" not in _NC_CACHE:
        _NC_CACHE["nc"] = build_nc()
    return _NC_CACHE["nc"]


def run_on_hw(inputs, trace=False):
    """Returns (out [1,T,D] fp32, BassKernelResults)."""
    nc = _get_nc()
    in_maps = host_prep(inputs["x"], inputs["WQ"], inputs["WK"],
                        inputs["WV"], inputs["WO"])
    res = run_bass_kernel_spmd(nc, in_maps, list(range(N_CORES)),
                               trace=trace)
    acc = np.zeros((T, D), dtype=np.float64)
    for c in range(N_CORES):
        acc += res.results[c]["out"].astype(np.float64)
    return acc.astype(np.float32)[None], res


def kernel(x, WQ, WK, WV, WO):
    out, _ = run_on_hw({"x": x, "WQ": WQ, "WK": WK, "WV": WV, "WO": WO})
    return out


# revision 9
# speedup vs baseline: 1.1676x; 1.1676x over previous
"""GroupedQueryAttention (B=1, T=4096, D=2048, 16 q-heads / 4 kv-heads, RoPE,
causal) on 8 Trainium2 NeuronCores — v6.

Sharding: head tensor-parallel, core c owns q-heads {2c, 2c+1} and kv head
c//2. Each core computes partial out = ctx_heads @ WO_slice over the full
sequence; bf16 partials are summed on the host.

v6 idea: ONE dense tensor-engine stream. All chunk-boundary PE work
(projection accums, RoPE perm, V transposes, normalization broadcast, WO
output tiles) is queued as filler closures and drained inside the
ACT-paced attention loops, so the PE never ping-pongs between a PE-only
projection window and an ACT-bound attention window (which left HAM
oscillating at low clock).

PSUM: psX 3x[128,512] (S tiles, attention-only) + psC 2 (ctx per head)
+ psL 1 (both heads' denominators at partitions 0/32, col-packed ones
matmuls) + psW 2 (all transient boundary tiles) = 8 banks.
"""

import math

import numpy as np
import ml_dtypes

import concourse.bass as bass
import concourse.mybir as mybir
import concourse.tile as tile
from concourse.bass_utils import run_bass_kernel_spmd

FP = mybir.dt.float32
BF = mybir.dt.bfloat16
BFNP = ml_dtypes.bfloat16

T, D, DH = 4096, 2048, 128      # seq len, model dim, head dim
CH = 512                        # query-chunk (free dim of attention matmuls)
N_CORES = 8


# --------------------------------------------------------------------------
# workaround: this walrus build rejects instructions carrying >1 sem-waits
# (setupSyncWait "Too many sync wait commands"); split extras into NoOps.
_WS_CTR = [0]


def _split_multi_waits(nc, limit=1):
    for f in nc.m.functions:
        for bb in f.blocks:
            il = bb.instructions
            i = 0
            while i < len(il):
                inst = il[i]
                si = getattr(inst, "sync_info", None)
                if si is not None and len(si.on_wait) > limit:
                    waits = list(si.on_wait)
                    keep, rest = waits[:limit], waits[limit:]
                    nops = []
                    for j in range(0, len(rest), limit):
                        _WS_CTR[0] += 1
                        n = mybir.InstNoOp(name=f"waitsplit-{_WS_CTR[0]}")
                        n.engine = inst.engine
                        n.sync_info = mybir.SyncInfo(
                            on_wait=rest[j:j + limit], on_update=[])
                        nops.append(n)
                    inst.sync_info = mybir.SyncInfo(
                        on_wait=keep, on_update=list(si.on_update))
                    for k, n in enumerate(nops):
                        il.insert(i + k, n)
                    i += len(nops)
                i += 1


# --------------------------------------------------------------------------
def build_nc():
    nT = T // CH          # 8 T-chunks
    nA = D // 128         # 16 contraction tiles
    nS = CH // 128        # 4 kt subtiles per chunk
    nN = D // 512         # 4 output column tiles
    ISQ = 1.0 / math.sqrt(float(DH))

    nc = bass.Bass()

    xT = nc.dram_tensor("xT", [D, T], BF, kind="ExternalInput")
    wq2 = nc.dram_tensor("wq2", [128, nA * 256], BF, kind="ExternalInput")
    wk2 = nc.dram_tensor("wk2", [128, nA * 128], BF, kind="ExternalInput")
    wv2 = nc.dram_tensor("wv2", [128, nA * 128], BF, kind="ExternalInput")
    wo2 = nc.dram_tensor("wo2", [128, 2 * D], BF, kind="ExternalInput")
    cosT = nc.dram_tensor("cosT", [128, T], BF, kind="ExternalInput")
    sinT = nc.dram_tensor("sinT", [128, T], BF, kind="ExternalInput")
    permM = nc.dram_tensor("permM", [128, 128], BF, kind="ExternalInput")
    masks = nc.dram_tensor("masks", [128, nS * CH], BF, kind="ExternalInput")
    onescol = nc.dram_tensor("onescol", [128, 1], BF, kind="ExternalInput")
    onesrow = nc.dram_tensor("onesrow", [64, 128], BF, kind="ExternalInput")
    ident = nc.dram_tensor("ident", [128, 128], BF, kind="ExternalInput")
    out = nc.dram_tensor("out", [T, D], BF, kind="ExternalOutput")

    xTr = xT.rearrange("(a p) t -> p a t", p=128)

    with tile.TileContext(nc) as tc:
        with (
            tc.tile_pool(name="res", bufs=1) as res,
            tc.tile_pool(name="ktv", bufs=2 * nT) as ktv,
            tc.tile_pool(name="xt", bufs=32) as xtp,
            tc.tile_pool(name="qk", bufs=4) as qkp,     # q/k/vT sbuf copies
            tc.tile_pool(name="qr", bufs=4) as qrp,     # rope outputs q heads
            tc.tile_pool(name="rt", bufs=4) as rtp,     # rope temporaries
            tc.tile_pool(name="pp", bufs=4) as ppp,     # P tiles
            tc.tile_pool(name="nrm", bufs=2) as nrm,    # recip / bcast / cn
            tc.tile_pool(name="oo", bufs=4) as oop,     # out staging bf16
            tc.tile_pool(name="psX", bufs=3, space="PSUM") as psX,
            tc.tile_pool(name="psC", bufs=2, space="PSUM") as psC,
            tc.tile_pool(name="psL", bufs=1, space="PSUM") as psL,
            tc.tile_pool(name="psW", bufs=2, space="PSUM") as psW,
        ):
            wq_sb = res.tile([128, nA * 256], BF, name="wq_sb")
            wk_sb = res.tile([128, nA * 128], BF, name="wk_sb")
            wv_sb = res.tile([128, nA * 128], BF, name="wv_sb")
            wo_sb = res.tile([128, 2 * D], BF, name="wo_sb")
            mask_sb = res.tile([128, nS * CH], BF, name="mask_sb")
            perm_sb = res.tile([128, 128], BF, name="perm_sb")
            oc_sb = res.tile([128, 1], BF, name="oc_sb")
            or_sb = res.tile([64, 128], BF, name="or_sb")
            id_sb = res.tile([128, 128], BF, name="id_sb")
            cos_sb = res.tile([128, T], BF, name="cos_sb")
            sin_sb = res.tile([128, T], BF, name="sin_sb")

            kt_tiles = {}
            v_tiles = {}
            x_chunks = {}
            qr_chunks = {}

            def load_x(qc):
                t0 = qc * CH
                xts = []
                for a in range(nA):
                    xa = xtp.tile([128, CH], BF, name=f"x{qc}_{a}", tag="xt")
                    nc.sync.dma_start(xa[:], xTr[:, a, t0:t0 + CH])
                    xts.append(xa)
                x_chunks[qc] = xts

            # x for chunk 0 first so projections can start ASAP, then wq,
            # then everything else.
            load_x(0)
            for t_, s_ in [
                (wq_sb, wq2), (wk_sb, wk2), (wv_sb, wv2), (cos_sb, cosT),
                (sin_sb, sinT), (perm_sb, permM), (mask_sb, masks),
                (oc_sb, onescol), (or_sb, onesrow), (id_sb, ident),
                (wo_sb, wo2),
            ]:
                nc.sync.dma_start(t_[:], s_[:])

            # ---------- projection / rope closures ----------
            def proj_closures(qc):
                """Filler closures, in dependency order, computing Q/K/V^T
                projections, V transposes, and RoPE for chunk qc."""
                t0 = qc * CH
                cos_t = cos_sb[:, t0:t0 + CH]
                sin_t = sin_sb[:, t0:t0 + CH]
                qr0 = qrp.tile([128, CH], BF, name=f"qr0_{qc}", tag="qr")
                qr1 = qrp.tile([128, CH], BF, name=f"qr1_{qc}", tag="qr")
                ktt = ktv.tile([128, CH], BF, name=f"kt{qc}", tag="ktv")
                qr_chunks[qc] = (qr0, qr1)
                kt_tiles[qc] = ktt
                vt = ktv.tile([128, nS * 128], BF, name=f"v{qc}", tag="ktv")
                v_tiles[qc] = vt

                sb_holder = {}

                def mk_accum(lhs, nm):
                    def emit():
                        xts = x_chunks[qc]
                        acc = psW.tile([128, CH], FP, name=f"ps_{nm}",
                                       tag="w")
                        for a in range(nA):
                            nc.tensor.matmul(
                                acc[:], lhs(a), xts[a][:],
                                start=(a == 0), stop=(a == nA - 1))
                        sb_ = qkp.tile([128, CH], BF, name=f"{nm}s",
                                       tag="qk")
                        nc.scalar.copy(sb_[:], acc[:])
                        sb_holder[nm] = sb_
                    return emit

                def mk_rope(nm, dst):
                    def emit():
                        src_sb = sb_holder[nm]
                        sw = psW.tile([128, CH], FP, name=f"sw_{nm}",
                                      tag="w")
                        nc.tensor.matmul(sw[:], perm_sb[:], src_sb[:])
                        t1 = rtp.tile([128, CH], BF, name=f"r1_{nm}",
                                      tag="rt")
                        nc.vector.tensor_mul(t1[:], src_sb[:], cos_t)
                        t2 = rtp.tile([128, CH], BF, name=f"r2_{nm}",
                                      tag="rt")
                        nc.vector.tensor_mul(t2[:], sw[:], sin_t)
                        nc.vector.tensor_add(dst[:], t1[:], t2[:])
                    return emit

                def mk_vtrans():
                    def emit():
                        vT_sb = sb_holder[f"vT{qc}"]
                        for s in range(nS):
                            tp = psW.tile([128, 128], BF, name=f"tp{qc}_{s}",
                                          tag="w", padded_shape=[128, 512])
                            nc.tensor.transpose(
                                tp[:], vT_sb[:, s * 128:(s + 1) * 128],
                                id_sb[:])
                            nc.vector.tensor_copy(
                                vt[:, s * 128:(s + 1) * 128], tp[:])
                    return emit

                cl = []
                cl.append(mk_accum(lambda a: wq_sb[:, a * 256:a * 256 + 128],
                                   f"q0{qc}"))
                cl.append(mk_rope(f"q0{qc}", qr0))
                cl.append(mk_accum(lambda a: wq_sb[:, a * 256 + 128:
                                                   a * 256 + 256],
                                   f"q1{qc}"))
                cl.append(mk_rope(f"q1{qc}", qr1))
                cl.append(mk_accum(lambda a: wk_sb[:, a * 128:(a + 1) * 128],
                                   f"k{qc}"))
                cl.append(mk_rope(f"k{qc}", ktt))
                cl.append(mk_accum(lambda a: wv_sb[:, a * 128:(a + 1) * 128],
                                   f"vT{qc}"))
                cl.append(mk_vtrans())
                return cl

            # ---------- normalization part 2 + WO closures ----------
            def norm2_wo_closures(qc, rb_t, ctx):
                t0 = qc * CH
                cns = []

                def mk_norm(h):
                    def emit():
                        bc_ps = psW.tile([128, CH], FP, name=f"bc{qc}_{h}",
                                         tag="w")
                        nc.tensor.matmul(bc_ps[:],
                                         or_sb[32 * h:32 * h + 1, :],
                                         rb_t[32 * h:32 * h + 1, :])
                        bc_sb = nrm.tile([128, CH], FP, name=f"bcs{qc}_{h}",
                                         tag="bc")
                        nc.scalar.copy(bc_sb[:], bc_ps[:])
                        cn = nrm.tile([128, CH], BF, name=f"cn{qc}_{h}",
                                      tag="cn")
                        nc.vector.tensor_mul(cn[:], ctx[h][:], bc_sb[:])
                        cns.append(cn)
                    return emit

                def mk_wo(s, n):
                    def emit():
                        w_ps = psW.tile([128, 512], FP,
                                        name=f"w{qc}_{s}_{n}", tag="w")
                        nc.tensor.matmul(
                            w_ps[:], cns[0][:, s * 128:(s + 1) * 128],
                            wo_sb[:, n * 512:(n + 1) * 512],
                            start=True, stop=False)
                        nc.tensor.matmul(
                            w_ps[:], cns[1][:, s * 128:(s + 1) * 128],
                            wo_sb[:, D + n * 512:D + (n + 1) * 512],
                            start=False, stop=True)
                        osb = oop.tile([128, 512], BF, name=f"o{qc}_{s}_{n}",
                                       tag="osb")
                        nc.vector.tensor_copy(osb[:], w_ps[:])
                        nc.sync.dma_start(
                            out[t0 + s * 128:t0 + (s + 1) * 128,
                                n * 512:(n + 1) * 512], osb[:])
                    return emit

                return ([mk_norm(0), mk_norm(1)] +
                        [mk_wo(s, n) for s in range(nS) for n in range(nN)])

            # ---------- attention ----------
            def emit_attention(qc, queue):
                """Causal attention for chunk qc; pops filler closures from
                `queue` inside the loop and drains it at the end."""
                nkt = (qc + 1) * nS
                qr0, qr1 = qr_chunks[qc]
                qrs = (qr0, qr1)

                ctx = [psC.tile([128, CH], FP, name=f"ctx{qc}_{h}", tag="ctx")
                       for h in range(2)]
                lt = psL.tile([128, CH], FP, name=f"l{qc}", tag="l")
                lps = [lt[0:1, :], lt[32:33, :]]

                def emit_s(kt, h):
                    kc, ko = kt // nS, (kt % nS) * 128
                    sp = psX.tile([128, CH], FP, name=f"S{qc}_{h}_{kt}",
                                  tag="s")
                    nc.tensor.matmul(sp[:], kt_tiles[kc][:, ko:ko + 128],
                                     qrs[h][:])
                    return sp

                def emit_exp(kt, h, sp):
                    pt = ppp.tile([128, CH], BF, name=f"P{qc}_{h}_{kt}",
                                  tag="p")
                    nc.scalar.activation(pt[:], sp[:],
                                         mybir.ActivationFunctionType.Exp,
                                         scale=ISQ)
                    delta = kt - qc * nS
                    if delta >= 0:  # diagonal chunk: causal mask
                        nc.vector.tensor_mul(
                            pt[:], pt[:],
                            mask_sb[:, delta * CH:(delta + 1) * CH])
                    return pt

                # pipeline: S/exp one step ahead; l pair adjacent (packed)
                p0 = emit_exp(0, 0, emit_s(0, 0))
                p1 = emit_exp(0, 1, emit_s(0, 1))
                for kt in range(nkt):
                    if kt + 1 < nkt:
                        pn0 = emit_exp(kt + 1, 0, emit_s(kt + 1, 0))
                        pn1 = emit_exp(kt + 1, 1, emit_s(kt + 1, 1))
                    st, sp_ = (kt == 0), (kt == nkt - 1)
                    nc.tensor.matmul(lps[0], oc_sb[:], p0[:],
                                     start=st, stop=sp_)
                    nc.tensor.matmul(lps[1], oc_sb[:], p1[:],
                                     start=st, stop=sp_)
                    kc, ko = kt // nS, (kt % nS) * 128
                    vt = v_tiles[kc]
                    nc.tensor.matmul(ctx[0][:], vt[:, ko:ko + 128], p0[:],
                                     start=st, stop=sp_)
                    nc.tensor.matmul(ctx[1][:], vt[:, ko:ko + 128], p1[:],
                                     start=st, stop=sp_)
                    if len(queue) > 5:   # hold a reserve for the chunk end
                        queue.pop(0)()
                    if kt + 1 < nkt:
                        p0, p1 = pn0, pn1

                # normalization part 1 (DVE), emitted BEFORE the drain so the
                # chain starts immediately: copy l out of PSUM (frees the bank
                # for the next chunk's accumulation ~4us earlier than letting
                # the reciprocal hold it), then reciprocal + bf16 cast on the
                # SBUF copy. The reserved fillers below keep the PE busy while
                # the chain completes.
                l_sb = nrm.tile([64, CH], FP, name=f"ls{qc}", tag="ls")
                nc.vector.tensor_copy(l_sb[:], lt[0:64, :])
                r_t = nrm.tile([64, CH], FP, name=f"r{qc}", tag="r")
                nc.vector.reciprocal(r_t[:], l_sb[:])
                rb_t = nrm.tile([64, CH], BF, name=f"rb{qc}", tag="rb")
                nc.vector.tensor_copy(rb_t[:], r_t[:])
                while queue:   # must drain: next chunk needs qr/kt ready
                    queue.pop(0)()
                return rb_t, ctx

            # ---------- main schedule ----------
            queue = []
            for cl in proj_closures(0):
                cl()
            for qc in range(nT):
                if qc + 1 < nT:
                    load_x(qc + 1)
                    queue.extend(proj_closures(qc + 1))
                rb_t, ctx = emit_attention(qc, queue)
                queue.extend(norm2_wo_closures(qc, rb_t, ctx))
            for cl in queue:
                cl()

    _split_multi_waits(nc, 1)
    return nc


# --------------------------------------------------------------------------
def host_prep(x, WQ, WK, WV, WO):
    nA = D // 128
    nS = CH // 128
    ROPE_BASE = 10000.0

    xTc = np.ascontiguousarray(
        np.asarray(x, dtype=np.float32).reshape(T, D).T).astype(BFNP)

    omega = 1.0 / (ROPE_BASE ** (np.arange(0, DH, 2, dtype=np.float64) / DH))
    ang = np.outer(omega, np.arange(T, dtype=np.float64))
    cosT = np.repeat(np.cos(ang), 2, axis=0).astype(BFNP)
    sgn = np.tile(np.array([-1.0, 1.0]), DH // 2)[:, None]
    sinT = (np.repeat(np.sin(ang), 2, axis=0) * sgn).astype(BFNP)

    permM = np.zeros((128, 128), dtype=np.float32)
    for j in range(0, 128, 2):
        permM[j + 1, j] = 1.0
        permM[j, j + 1] = 1.0
    permM = permM.astype(BFNP)

    p_i = np.arange(128)[:, None]
    f_i = np.arange(CH)[None, :]
    masks = np.concatenate(
        [(128 * dl + p_i <= f_i).astype(np.float32) for dl in range(nS)],
        axis=1).astype(BFNP)

    def tile_pmaj(w, ncols):
        return np.ascontiguousarray(
            np.asarray(w, dtype=np.float32).reshape(nA, 128, ncols)
            .transpose(1, 0, 2).reshape(128, nA * ncols)).astype(BFNP)

    in_maps = []
    for c in range(N_CORES):
        kv = c // 2
        wo_c = np.asarray(WO, dtype=np.float32)[256 * c:256 * (c + 1), :]
        in_maps.append({
            "xT": xTc,
            "wq2": tile_pmaj(np.asarray(WQ)[:, 256 * c:256 * (c + 1)], 256),
            "wk2": tile_pmaj(np.asarray(WK)[:, 128 * kv:128 * (kv + 1)], 128),
            "wv2": tile_pmaj(np.asarray(WV)[:, 128 * kv:128 * (kv + 1)], 128),
            "wo2": np.ascontiguousarray(
                wo_c.reshape(2, 128, D).transpose(1, 0, 2)
                .reshape(128, 2 * D)).astype(BFNP),
            "cosT": cosT, "sinT": sinT, "permM": permM, "masks": masks,
            "onescol": np.ones((128, 1), dtype=BFNP),
            "onesrow": np.ones((64, 128), dtype=BFNP),
            "ident": np.eye(128, dtype=np.float32).astype(BFNP),
        })
    return in_maps


_NC_CACHE = {}


def _get_nc():
    if "nc" not in _NC_CACHE:
        _NC_CACHE["nc"] = build_nc()
    return _NC_CACHE["nc"]


def run_on_hw(inputs, trace=False):
    """Returns (out [1,T,D] fp32, BassKernelResults)."""
    nc = _get_nc()
    in_maps = host_prep(inputs["x"], inputs["WQ"], inputs["WK"],
                        inputs["WV"], inputs["WO"])
    res = run_bass_kernel_spmd(nc, in_maps, list(range(N_CORES)),
                               trace=trace)
    acc = np.zeros((T, D), dtype=np.float64)
    for c in range(N_CORES):
        acc += res.results[c]["out"].astype(np.float64)
    return acc.astype(np.float32)[None], res


def kernel(x, WQ, WK, WV, WO):
    out, _ = run_on_hw({"x": x, "WQ": WQ, "WK": WK, "WV": WV, "WO": WO})
    return out


# revision 10
# speedup vs baseline: 1.2122x; 1.0382x over previous
"""GroupedQueryAttention (B=1, T=4096, D=2048, 16 q-heads / 4 kv-heads, RoPE,
causal) on 8 Trainium2 NeuronCores — v6.

Sharding: head tensor-parallel, core c owns q-heads {2c, 2c+1} and kv head
c//2. Each core computes partial out = ctx_heads @ WO_slice over the full
sequence; bf16 partials are summed on the host.

v6 idea: ONE dense tensor-engine stream. All chunk-boundary PE work
(projection accums, RoPE perm, V transposes, normalization broadcast, WO
output tiles) is queued as filler closures and drained inside the
ACT-paced attention loops, so the PE never ping-pongs between a PE-only
projection window and an ACT-bound attention window (which left HAM
oscillating at low clock).

PSUM: psX 3x[128,512] (S tiles, attention-only) + psC 2 (ctx per head)
+ psL 1 (both heads' denominators at partitions 0/32, col-packed ones
matmuls) + psW 2 (all transient boundary tiles) = 8 banks.
"""

import math

import numpy as np
import ml_dtypes

import concourse.bass as bass
import concourse.mybir as mybir
import concourse.tile as tile
from concourse.bass_utils import run_bass_kernel_spmd

FP = mybir.dt.float32
BF = mybir.dt.bfloat16
BFNP = ml_dtypes.bfloat16

T, D, DH = 4096, 2048, 128      # seq len, model dim, head dim
CH = 512                        # query-chunk (free dim of attention matmuls)
N_CORES = 8


# --------------------------------------------------------------------------
# workaround: this walrus build rejects instructions carrying >1 sem-waits
# (setupSyncWait "Too many sync wait commands"); split extras into NoOps.
_WS_CTR = [0]


def _split_multi_waits(nc, limit=1):
    for f in nc.m.functions:
        for bb in f.blocks:
            il = bb.instructions
            i = 0
            while i < len(il):
                inst = il[i]
                si = getattr(inst, "sync_info", None)
                if si is not None and len(si.on_wait) > limit:
                    waits = list(si.on_wait)
                    keep, rest = waits[:limit], waits[limit:]
                    nops = []
                    for j in range(0, len(rest), limit):
                        _WS_CTR[0] += 1
                        n = mybir.InstNoOp(name=f"waitsplit-{_WS_CTR[0]}")
                        n.engine = inst.engine
                        n.sync_info = mybir.SyncInfo(
                            on_wait=rest[j:j + limit], on_update=[])
                        nops.append(n)
                    inst.sync_info = mybir.SyncInfo(
                        on_wait=keep, on_update=list(si.on_update))
                    for k, n in enumerate(nops):
                        il.insert(i + k, n)
                    i += len(nops)
                i += 1


# --------------------------------------------------------------------------
def build_nc():
    nT = T // CH          # 8 T-chunks
    nA = D // 128         # 16 contraction tiles
    nS = CH // 128        # 4 kt subtiles per chunk
    nN = D // 512         # 4 output column tiles
    ISQ = 1.0 / math.sqrt(float(DH))

    nc = bass.Bass()

    xT = nc.dram_tensor("xT", [D, T], BF, kind="ExternalInput")
    wq2 = nc.dram_tensor("wq2", [128, nA * 256], BF, kind="ExternalInput")
    wk2 = nc.dram_tensor("wk2", [128, nA * 128], BF, kind="ExternalInput")
    wv2 = nc.dram_tensor("wv2", [128, nA * 128], BF, kind="ExternalInput")
    wo2 = nc.dram_tensor("wo2", [128, 2 * D], BF, kind="ExternalInput")
    cosT = nc.dram_tensor("cosT", [128, T], BF, kind="ExternalInput")
    sinT = nc.dram_tensor("sinT", [128, T], BF, kind="ExternalInput")
    permM = nc.dram_tensor("permM", [128, 128], BF, kind="ExternalInput")
    masks = nc.dram_tensor("masks", [128, nS * CH], BF, kind="ExternalInput")
    onescol = nc.dram_tensor("onescol", [128, 1], BF, kind="ExternalInput")
    onesrow = nc.dram_tensor("onesrow", [64, 128], BF, kind="ExternalInput")
    ident = nc.dram_tensor("ident", [128, 128], BF, kind="ExternalInput")
    out = nc.dram_tensor("out", [T, D], BF, kind="ExternalOutput")

    xTr = xT.rearrange("(a p) t -> p a t", p=128)

    with tile.TileContext(nc) as tc:
        with (
            tc.tile_pool(name="res", bufs=1) as res,
            tc.tile_pool(name="ktv", bufs=2 * nT) as ktv,
            tc.tile_pool(name="xt", bufs=32) as xtp,
            tc.tile_pool(name="qk", bufs=4) as qkp,     # q/k/vT sbuf copies
            tc.tile_pool(name="qr", bufs=4) as qrp,     # rope outputs q heads
            tc.tile_pool(name="rt", bufs=4) as rtp,     # rope temporaries
            tc.tile_pool(name="pp", bufs=4) as ppp,     # P tiles
            tc.tile_pool(name="nrm", bufs=2) as nrm,    # recip / bcast / cn
            tc.tile_pool(name="oo", bufs=4) as oop,     # out staging bf16
            tc.tile_pool(name="psX", bufs=3, space="PSUM") as psX,
            tc.tile_pool(name="psC", bufs=2, space="PSUM") as psC,
            tc.tile_pool(name="psL", bufs=1, space="PSUM") as psL,
            tc.tile_pool(name="psW", bufs=2, space="PSUM") as psW,
        ):
            wq_sb = res.tile([128, nA * 256], BF, name="wq_sb")
            wk_sb = res.tile([128, nA * 128], BF, name="wk_sb")
            wv_sb = res.tile([128, nA * 128], BF, name="wv_sb")
            wo_sb = res.tile([128, 2 * D], BF, name="wo_sb")
            mask_sb = res.tile([128, nS * CH], BF, name="mask_sb")
            perm_sb = res.tile([128, 128], BF, name="perm_sb")
            oc_sb = res.tile([128, 1], BF, name="oc_sb")
            or_sb = res.tile([64, 128], BF, name="or_sb")
            id_sb = res.tile([128, 128], BF, name="id_sb")
            cos_sb = res.tile([128, T], BF, name="cos_sb")
            sin_sb = res.tile([128, T], BF, name="sin_sb")

            kt_tiles = {}
            v_tiles = {}
            x_chunks = {}
            qr_chunks = {}

            def load_x(qc):
                t0 = qc * CH
                xts = []
                for a in range(nA):
                    xa = xtp.tile([128, CH], BF, name=f"x{qc}_{a}", tag="xt")
                    nc.sync.dma_start(xa[:], xTr[:, a, t0:t0 + CH])
                    xts.append(xa)
                x_chunks[qc] = xts

            # x for chunk 0 first so projections can start ASAP, then wq,
            # then everything else.
            load_x(0)
            for t_, s_ in [
                (wq_sb, wq2), (wk_sb, wk2), (wv_sb, wv2), (cos_sb, cosT),
                (sin_sb, sinT), (perm_sb, permM), (mask_sb, masks),
                (oc_sb, onescol), (or_sb, onesrow), (id_sb, ident),
                (wo_sb, wo2),
            ]:
                nc.sync.dma_start(t_[:], s_[:])

            # ---------- projection / rope closures ----------
            def proj_closures(qc):
                """Filler closures, in dependency order, computing Q/K/V^T
                projections, V transposes, and RoPE for chunk qc."""
                t0 = qc * CH
                cos_t = cos_sb[:, t0:t0 + CH]
                sin_t = sin_sb[:, t0:t0 + CH]
                qr0 = qrp.tile([128, CH], BF, name=f"qr0_{qc}", tag="qr")
                qr1 = qrp.tile([128, CH], BF, name=f"qr1_{qc}", tag="qr")
                ktt = ktv.tile([128, CH], BF, name=f"kt{qc}", tag="ktv")
                qr_chunks[qc] = (qr0, qr1)
                kt_tiles[qc] = ktt
                vt = ktv.tile([128, nS * 128], BF, name=f"v{qc}", tag="ktv")
                v_tiles[qc] = vt

                sb_holder = {}

                def mk_accum(lhs, nm):
                    def emit():
                        xts = x_chunks[qc]
                        acc = psW.tile([128, CH], FP, name=f"ps_{nm}",
                                       tag="w")
                        for a in range(nA):
                            nc.tensor.matmul(
                                acc[:], lhs(a), xts[a][:],
                                start=(a == 0), stop=(a == nA - 1))
                        sb_ = qkp.tile([128, CH], BF, name=f"{nm}s",
                                       tag="qk")
                        nc.scalar.copy(sb_[:], acc[:])
                        sb_holder[nm] = sb_
                    return emit

                def mk_rope(nm, dst):
                    def emit():
                        src_sb = sb_holder[nm]
                        sw = psW.tile([128, CH], FP, name=f"sw_{nm}",
                                      tag="w")
                        nc.tensor.matmul(sw[:], perm_sb[:], src_sb[:])
                        t1 = rtp.tile([128, CH], BF, name=f"r1_{nm}",
                                      tag="rt")
                        nc.vector.tensor_mul(t1[:], src_sb[:], cos_t)
                        t2 = rtp.tile([128, CH], BF, name=f"r2_{nm}",
                                      tag="rt")
                        nc.vector.tensor_mul(t2[:], sw[:], sin_t)
                        nc.vector.tensor_add(dst[:], t1[:], t2[:])
                    return emit

                def mk_vtrans():
                    def emit():
                        vT_sb = sb_holder[f"vT{qc}"]
                        for s in range(nS):
                            tp = psW.tile([128, 128], BF, name=f"tp{qc}_{s}",
                                          tag="w", padded_shape=[128, 512])
                            nc.tensor.transpose(
                                tp[:], vT_sb[:, s * 128:(s + 1) * 128],
                                id_sb[:])
                            nc.vector.tensor_copy(
                                vt[:, s * 128:(s + 1) * 128], tp[:])
                    return emit

                cl = []
                cl.append(mk_accum(lambda a: wq_sb[:, a * 256:a * 256 + 128],
                                   f"q0{qc}"))
                cl.append(mk_rope(f"q0{qc}", qr0))
                cl.append(mk_accum(lambda a: wq_sb[:, a * 256 + 128:
                                                   a * 256 + 256],
                                   f"q1{qc}"))
                cl.append(mk_rope(f"q1{qc}", qr1))
                cl.append(mk_accum(lambda a: wk_sb[:, a * 128:(a + 1) * 128],
                                   f"k{qc}"))
                cl.append(mk_rope(f"k{qc}", ktt))
                cl.append(mk_accum(lambda a: wv_sb[:, a * 128:(a + 1) * 128],
                                   f"vT{qc}"))
                cl.append(mk_vtrans())
                return cl

            # ---------- normalization part 2 + WO closures ----------
            def norm2_wo_closures(qc, rb_t, ctx):
                t0 = qc * CH
                cns = []

                def mk_norm(h):
                    def emit():
                        bc_ps = psW.tile([128, CH], FP, name=f"bc{qc}_{h}",
                                         tag="w")
                        nc.tensor.matmul(bc_ps[:],
                                         or_sb[32 * h:32 * h + 1, :],
                                         rb_t[32 * h:32 * h + 1, :])
                        bc_sb = nrm.tile([128, CH], FP, name=f"bcs{qc}_{h}",
                                         tag="bc")
                        nc.scalar.copy(bc_sb[:], bc_ps[:])
                        cn = nrm.tile([128, CH], BF, name=f"cn{qc}_{h}",
                                      tag="cn")
                        nc.vector.tensor_mul(cn[:], ctx[h][:], bc_sb[:])
                        cns.append(cn)
                    return emit

                def mk_wo(s, n):
                    def emit():
                        w_ps = psW.tile([128, 512], FP,
                                        name=f"w{qc}_{s}_{n}", tag="w")
                        nc.tensor.matmul(
                            w_ps[:], cns[0][:, s * 128:(s + 1) * 128],
                            wo_sb[:, n * 512:(n + 1) * 512],
                            start=True, stop=False)
                        nc.tensor.matmul(
                            w_ps[:], cns[1][:, s * 128:(s + 1) * 128],
                            wo_sb[:, D + n * 512:D + (n + 1) * 512],
                            start=False, stop=True)
                        osb = oop.tile([128, 512], BF, name=f"o{qc}_{s}_{n}",
                                       tag="osb")
                        nc.vector.tensor_copy(osb[:], w_ps[:])
                        nc.sync.dma_start(
                            out[t0 + s * 128:t0 + (s + 1) * 128,
                                n * 512:(n + 1) * 512], osb[:])
                    return emit

                return ([mk_norm(0), mk_norm(1)] +
                        [mk_wo(s, n) for s in range(nS) for n in range(nN)])

            # ---------- attention ----------
            def emit_attention(qc, queue):
                """Causal attention for chunk qc; pops filler closures from
                `queue` inside the loop and drains it at the end."""
                nkt = (qc + 1) * nS
                qr0, qr1 = qr_chunks[qc]
                qrs = (qr0, qr1)

                ctx = [psC.tile([128, CH], FP, name=f"ctx{qc}_{h}", tag="ctx")
                       for h in range(2)]
                lt = psL.tile([128, CH], FP, name=f"l{qc}", tag="l")
                lps = [lt[0:1, :], lt[32:33, :]]

                def emit_s(kt, h):
                    # causal trim: a diagonal key-block at offset 128*delta
                    # is only valid for q >= 128*delta — stream only those
                    # columns (saves 37.5% of diagonal-tile rows on S/l/AV).
                    kc, ko = kt // nS, (kt % nS) * 128
                    delta = kt - qc * nS
                    q0 = 128 * delta if delta > 0 else 0
                    sp = psX.tile([128, CH - q0], FP,
                                  name=f"S{qc}_{h}_{kt}", tag="s",
                                  padded_shape=[128, CH])
                    nc.tensor.matmul(sp[:], kt_tiles[kc][:, ko:ko + 128],
                                     qrs[h][:, q0:CH])
                    return sp, q0

                def emit_exp(kt, h, sp, q0):
                    pt = ppp.tile([128, CH - q0], BF,
                                  name=f"P{qc}_{h}_{kt}", tag="p",
                                  padded_shape=[128, CH])
                    nc.scalar.activation(pt[:], sp[:],
                                         mybir.ActivationFunctionType.Exp,
                                         scale=ISQ)
                    delta = kt - qc * nS
                    if delta >= 0:
                        # only the first 128 trimmed columns are triangular;
                        # the rest are fully valid
                        nc.vector.tensor_mul(
                            pt[:, 0:128], pt[:, 0:128],
                            mask_sb[:, delta * CH + q0:
                                    delta * CH + q0 + 128])
                    return pt

                # pipeline: S/exp one step ahead; l pair adjacent (packed)
                s0, q00 = emit_s(0, 0)
                s1, q01 = emit_s(0, 1)
                p0 = emit_exp(0, 0, s0, q00)
                p1 = emit_exp(0, 1, s1, q01)
                qq0, qq1 = q00, q01
                for kt in range(nkt):
                    if kt + 1 < nkt:
                        sn0, qn0 = emit_s(kt + 1, 0)
                        sn1, qn1 = emit_s(kt + 1, 1)
                        pn0 = emit_exp(kt + 1, 0, sn0, qn0)
                        pn1 = emit_exp(kt + 1, 1, sn1, qn1)
                    st, sp_ = (kt == 0), (kt == nkt - 1)
                    nc.tensor.matmul(lps[0][:, qq0:CH], oc_sb[:], p0[:],
                                     start=st, stop=sp_)
                    nc.tensor.matmul(lps[1][:, qq1:CH], oc_sb[:], p1[:],
                                     start=st, stop=sp_)
                    kc, ko = kt // nS, (kt % nS) * 128
                    vt = v_tiles[kc]
                    nc.tensor.matmul(ctx[0][:, qq0:CH], vt[:, ko:ko + 128],
                                     p0[:], start=st, stop=sp_)
                    nc.tensor.matmul(ctx[1][:, qq1:CH], vt[:, ko:ko + 128],
                                     p1[:], start=st, stop=sp_)
                    if len(queue) > 5:   # hold a reserve for the chunk end
                        queue.pop(0)()
                    if kt + 1 < nkt:
                        p0, p1 = pn0, pn1
                        qq0, qq1 = qn0, qn1

                # normalization part 1 (DVE), emitted BEFORE the drain so the
                # chain starts immediately: copy l out of PSUM (frees the bank
                # for the next chunk's accumulation ~4us earlier than letting
                # the reciprocal hold it), then reciprocal + bf16 cast on the
                # SBUF copy. The reserved fillers below keep the PE busy while
                # the chain completes.
                l_sb = nrm.tile([64, CH], FP, name=f"ls{qc}", tag="ls")
                nc.vector.tensor_copy(l_sb[:], lt[0:64, :])
                r_t = nrm.tile([64, CH], FP, name=f"r{qc}", tag="r")
                nc.vector.reciprocal(r_t[:], l_sb[:])
                rb_t = nrm.tile([64, CH], BF, name=f"rb{qc}", tag="rb")
                nc.vector.tensor_copy(rb_t[:], r_t[:])
                while queue:   # must drain: next chunk needs qr/kt ready
                    queue.pop(0)()
                return rb_t, ctx

            # ---------- main schedule ----------
            queue = []
            for cl in proj_closures(0):
                cl()
            for qc in range(nT):
                if qc + 1 < nT:
                    load_x(qc + 1)
                    queue.extend(proj_closures(qc + 1))
                rb_t, ctx = emit_attention(qc, queue)
                queue.extend(norm2_wo_closures(qc, rb_t, ctx))
            for cl in queue:
                cl()

    _split_multi_waits(nc, 1)
    return nc


# --------------------------------------------------------------------------
def host_prep(x, WQ, WK, WV, WO):
    nA = D // 128
    nS = CH // 128
    ROPE_BASE = 10000.0

    xTc = np.ascontiguousarray(
        np.asarray(x, dtype=np.float32).reshape(T, D).T).astype(BFNP)

    omega = 1.0 / (ROPE_BASE ** (np.arange(0, DH, 2, dtype=np.float64) / DH))
    ang = np.outer(omega, np.arange(T, dtype=np.float64))
    cosT = np.repeat(np.cos(ang), 2, axis=0).astype(BFNP)
    sgn = np.tile(np.array([-1.0, 1.0]), DH // 2)[:, None]
    sinT = (np.repeat(np.sin(ang), 2, axis=0) * sgn).astype(BFNP)

    permM = np.zeros((128, 128), dtype=np.float32)
    for j in range(0, 128, 2):
        permM[j + 1, j] = 1.0
        permM[j, j + 1] = 1.0
    permM = permM.astype(BFNP)

    p_i = np.arange(128)[:, None]
    f_i = np.arange(CH)[None, :]
    masks = np.concatenate(
        [(128 * dl + p_i <= f_i).astype(np.float32) for dl in range(nS)],
        axis=1).astype(BFNP)

    def tile_pmaj(w, ncols):
        return np.ascontiguousarray(
            np.asarray(w, dtype=np.float32).reshape(nA, 128, ncols)
            .transpose(1, 0, 2).reshape(128, nA * ncols)).astype(BFNP)

    in_maps = []
    for c in range(N_CORES):
        kv = c // 2
        wo_c = np.asarray(WO, dtype=np.float32)[256 * c:256 * (c + 1), :]
        in_maps.append({
            "xT": xTc,
            "wq2": tile_pmaj(np.asarray(WQ)[:, 256 * c:256 * (c + 1)], 256),
            "wk2": tile_pmaj(np.asarray(WK)[:, 128 * kv:128 * (kv + 1)], 128),
            "wv2": tile_pmaj(np.asarray(WV)[:, 128 * kv:128 * (kv + 1)], 128),
            "wo2": np.ascontiguousarray(
                wo_c.reshape(2, 128, D).transpose(1, 0, 2)
                .reshape(128, 2 * D)).astype(BFNP),
            "cosT": cosT, "sinT": sinT, "permM": permM, "masks": masks,
            "onescol": np.ones((128, 1), dtype=BFNP),
            "onesrow": np.ones((64, 128), dtype=BFNP),
            "ident": np.eye(128, dtype=np.float32).astype(BFNP),
        })
    return in_maps


_NC_CACHE = {}


def _get_nc():
    if "nc" not in _NC_CACHE:
        _NC_CACHE["nc"] = build_nc()
    return _NC_CACHE["nc"]


def run_on_hw(inputs, trace=False):
    """Returns (out [1,T,D] fp32, BassKernelResults)."""
    nc = _get_nc()
    in_maps = host_prep(inputs["x"], inputs["WQ"], inputs["WK"],
                        inputs["WV"], inputs["WO"])
    res = run_bass_kernel_spmd(nc, in_maps, list(range(N_CORES)),
                               trace=trace)
    acc = np.zeros((T, D), dtype=np.float64)
    for c in range(N_CORES):
        acc += res.results[c]["out"].astype(np.float64)
    return acc.astype(np.float32)[None], res


def kernel(x, WQ, WK, WV, WO):
    out, _ = run_on_hw({"x": x, "WQ": WQ, "WK": WK, "WV": WV, "WO": WO})
    return out
